# revision 3
# baseline (speedup 1.0000x reference)
"""MMoE layer kernel for 8 Trainium2 NeuronCores.

Reference math (B=4096, D=1024, H1=2048, H2=1024, E=7 experts, NS=7 scenes):
  h        = relu(einsum('bd,edh', x, W1) + b1)           # [B,E,H1]
  eo       = relu(einsum('beh,eho', h, W2) + b2)          # [B,E,H2]
  xc       = concat(x, scene_emb[scene])                  # [B, D+16]
  G        = softmax over s of einsum('bd,sde', xc, S)    # [B,E,NS] (after transpose)
  q        = mean_s log(G*7)                              # [B,E]
  score1   = logG[b, e, scene_b]
  select   = drop expert e iff e == argmin_e score1 == argmin_e q
  gate     = softmax_e(G[b,e,scene_b]) * select
  out      = einsum('be,beo', gate, eo); output = stack([out, out])

Sharding: data-parallel over batch (512 rows/core), weights replicated.
Expert MLP matmuls run in bf16 (fp32 accumulation in PSUM); all routing
math stays fp32 so the argmin/select decisions are bit-stable.

Device decomposition of the routing (no cross-partition broadcasts):
  Gpre[b, e*7+s] = x[b] @ Sflat + SE_table[scene_b]   (SE_table = scene_emb @ S[:,D:,:])
  Z = sum_s exp(Gpre); logZ = ln Z; SG = sum_s Gpre
  q      = SG/7 - logZ            (+const, argmin only)
  score1 = sum_s Gpre*onehot_s(scene) - logZ
  gate0  = softmax_e(exp(score1)) (logits in (0,1): no max-subtract needed)
  sel    = 1 - ismin(score1)*ismin(q)
  gate   = gate0 * sel
"""

import sys

if "/opt/trn_rl_repo" not in sys.path:
    sys.path.insert(0, "/opt/trn_rl_repo")

from contextlib import ExitStack

import ml_dtypes
import numpy as np

import concourse.bass as bass
import concourse.tile as tile
from concourse import bacc, mybir
from concourse.bass_utils import run_bass_kernel_spmd

F32 = mybir.dt.float32
BF16 = mybir.dt.bfloat16
AF = mybir.ActivationFunctionType
ALU = mybir.AluOpType
AX = mybir.AxisListType

N_CORES = 8
B, D, H1, H2, E, NS, T = 4096, 1024, 2048, 1024, 7, 7, 2
BL = B // N_CORES          # 512 rows per core
NB = BL // 128             # 4 batch tiles
KT1 = D // 128             # 8  k-tiles, layer 1
MT1 = H1 // 128            # 16 m-tiles, layer 1
KT2 = H1 // 128            # 16 k-tiles, layer 2
NO = H2 // 512             # 2  512-wide out column blocks
EN = E * NS                # 49
NP_BF16 = np.dtype(ml_dtypes.bfloat16)


def _emit_kernel(tc, aps, has_b1, has_b2):
    nc = tc.nc
    ctx = ExitStack()
    with ctx:
        consts = ctx.enter_context(tc.tile_pool(name="consts", bufs=1))

        # ---- persistent SBUF state -------------------------------------
        xtb_sb = consts.tile([128, KT1, BL], BF16)
        nc.sync.dma_start(xtb_sb[:, :, :], aps["xTb"].rearrange("(t p) b -> p t b", p=128))
        gate_sb = consts.tile([128, NB, E], F32)
        acc_sb = consts.tile([128, NB, H2], F32)
        if has_b1:
            b1_sb = consts.tile([128, E * MT1], F32)
            nc.sync.dma_start(b1_sb[:, :], aps["b1t"][:, :])
        if has_b2:
            b2_sb = consts.tile([1, E * H2], BF16)
            nc.sync.dma_start(b2_sb[:, :], aps["b2f"][:, :])
            ones_sb = consts.tile([1, 128], BF16)
            nc.vector.memset(ones_sb[:, :], 1.0)

        # ---- routing (fp32), small scratch released afterwards ---------
        rpool = tc.alloc_tile_pool(name="routing", bufs=1)
        rps = tc.alloc_tile_pool(name="rps", bufs=2, space="PSUM")

        xt_sb = rpool.tile([128, KT1, BL], F32)
        nc.sync.dma_start(xt_sb[:, :, :], aps["xT"].rearrange("(t p) b -> p t b", p=128))
        sflat_sb = rpool.tile([128, KT1, EN], F32)
        nc.sync.dma_start(sflat_sb[:, :, :], aps["sflat"].rearrange("(t p) j -> p t j", p=128))
        sett_sb = rpool.tile([10, EN], F32)
        nc.sync.dma_start(sett_sb[:, :], aps["sett"][:, :])
        scol_sb = rpool.tile([128, NB], F32)
        nc.sync.dma_start(scol_sb[:, :], aps["scol"][:, :])
        srow10_sb = rpool.tile([10, BL], F32)
        nc.sync.dma_start(srow10_sb[:, :], aps["srow"].to_broadcast((10, BL)))
        io7_sb = rpool.tile([128, EN], F32)
        nc.sync.dma_start(io7_sb[:, :], aps["iota7"].to_broadcast((128, EN)))
        io10_sb = rpool.tile([10, 1], F32)
        nc.sync.dma_start(io10_sb[:, :], aps["iota10"][:, :])

        # onehot over embedding rows, [10, BL]: onehot[r, b] = (scene[b] == r)
        onehot_sb = rpool.tile([10, BL], F32)
        nc.vector.tensor_scalar(
            out=onehot_sb[:, :], in0=srow10_sb[:, :],
            scalar1=io10_sb[:, 0:1], scalar2=None, op0=ALU.is_equal,
        )

        for t in range(NB):
            psr = rps.tile([128, EN], F32, tag="rps")
            for kt in range(KT1):
                nc.tensor.matmul(
                    psr[:, :],
                    lhsT=xt_sb[:, kt, bass.ts(t, 128)],
                    rhs=sflat_sb[:, kt, :],
                    start=(kt == 0), stop=False,
                )
            nc.tensor.matmul(
                psr[:, :],
                lhsT=onehot_sb[:, bass.ts(t, 128)],
                rhs=sett_sb[:, :],
                start=False, stop=True,
            )
            gp = rpool.tile([128, EN], F32, tag="gp")
            nc.scalar.copy(gp[:, :], psr[:, :])
            gp3 = gp.rearrange("p (e s) -> p e s", s=NS)

            eex = rpool.tile([128, EN], F32, tag="eex")
            nc.scalar.activation(eex[:, :], gp[:, :], AF.Exp)
            z = rpool.tile([128, E], F32, tag="z")
            nc.vector.tensor_reduce(out=z[:, :], in_=eex.rearrange("p (e s) -> p e s", s=NS), axis=AX.X, op=ALU.add)
            logz = rpool.tile([128, E], F32, tag="logz")
            nc.scalar.activation(logz[:, :], z[:, :], AF.Ln)
            sg = rpool.tile([128, E], F32, tag="sg")
            nc.vector.tensor_reduce(out=sg[:, :], in_=gp3, axis=AX.X, op=ALU.add)
            q = rpool.tile([128, E], F32, tag="q")
            nc.vector.scalar_tensor_tensor(
                out=q[:, :], in0=sg[:, :], scalar=1.0 / NS, in1=logz[:, :],
                op0=ALU.mult, op1=ALU.subtract,
            )
            oh49 = rpool.tile([128, EN], F32, tag="oh49")
            nc.vector.tensor_scalar(
                out=oh49[:, :], in0=io7_sb[:, :],
                scalar1=scol_sb[:, t : t + 1], scalar2=None, op0=ALU.is_equal,
            )
            gsel = rpool.tile([128, EN], F32, tag="gsel")
            nc.vector.tensor_tensor(out=gsel[:, :], in0=gp[:, :], in1=oh49[:, :], op=ALU.mult)
            s1s = rpool.tile([128, E], F32, tag="s1s")
            nc.vector.tensor_reduce(out=s1s[:, :], in_=gsel.rearrange("p (e s) -> p e s", s=NS), axis=AX.X, op=ALU.add)
            score1 = rpool.tile([128, E], F32, tag="score1")
            nc.vector.tensor_tensor(out=score1[:, :], in0=s1s[:, :], in1=logz[:, :], op=ALU.subtract)

            lg = rpool.tile([128, E], F32, tag="lg")
            nc.scalar.activation(lg[:, :], score1[:, :], AF.Exp)     # G at scene, in (0,1)
            el = rpool.tile([128, E], F32, tag="el")
            nc.scalar.activation(el[:, :], lg[:, :], AF.Exp)         # softmax numerator
            ssum = rpool.tile([128, 1], F32, tag="ssum")
            nc.vector.tensor_reduce(out=ssum[:, :], in_=el[:, :], axis=AX.X, op=ALU.add)
            rs = rpool.tile([128, 1], F32, tag="rs")
            nc.vector.reciprocal(rs[:, :], ssum[:, :])

            m1 = rpool.tile([128, 1], F32, tag="m1")
            nc.vector.tensor_reduce(out=m1[:, :], in_=score1[:, :], axis=AX.X, op=ALU.min)
            m2 = rpool.tile([128, 1], F32, tag="m2")
            nc.vector.tensor_reduce(out=m2[:, :], in_=q[:, :], axis=AX.X, op=ALU.min)
            k1 = rpool.tile([128, E], F32, tag="k1")
            nc.vector.tensor_scalar(
                out=k1[:, :], in0=score1[:, :], scalar1=m1[:, 0:1], scalar2=None, op0=ALU.is_equal,
            )
            k2 = rpool.tile([128, E], F32, tag="k2")
            nc.vector.tensor_scalar(
                out=k2[:, :], in0=q[:, :], scalar1=m2[:, 0:1], scalar2=None, op0=ALU.is_equal,
            )
            kill = rpool.tile([128, E], F32, tag="kill")
            nc.vector.tensor_tensor(out=kill[:, :], in0=k1[:, :], in1=k2[:, :], op=ALU.mult)
            sel = rpool.tile([128, E], F32, tag="sel")
            nc.vector.tensor_scalar(
                out=sel[:, :], in0=kill[:, :], scalar1=-1.0, scalar2=1.0,
                op0=ALU.mult, op1=ALU.add,
            )
            g0 = rpool.tile([128, E], F32, tag="g0")
            nc.vector.tensor_scalar(
                out=g0[:, :], in0=el[:, :], scalar1=rs[:, 0:1], scalar2=None, op0=ALU.mult,
            )
            nc.vector.tensor_tensor(out=gate_sb[:, t, :], in0=g0[:, :], in1=sel[:, :], op=ALU.mult)

        rps.release()
        rpool.release()

        # ---- expert MLPs (bf16 matmuls, fp32 accumulation) -------------
        w1pool = ctx.enter_context(tc.tile_pool(name="w1", bufs=2))
        w2pool = ctx.enter_context(tc.tile_pool(name="w2", bufs=2))
        htpool = ctx.enter_context(tc.tile_pool(name="ht", bufs=1))
        tmppool = ctx.enter_context(tc.tile_pool(name="tmp", bufs=3))
        l1ps = ctx.enter_context(tc.tile_pool(name="l1ps", bufs=4, space="PSUM"))
        l2ps = ctx.enter_context(tc.tile_pool(name="l2ps", bufs=3, space="PSUM"))

        for e in range(E):
            w1_sb = w1pool.tile([128, KT1, H1], BF16, tag="w1")
            nc.sync.dma_start(
                w1_sb[:, :, :], aps["w1"][e].rearrange("(t p) h -> p t h", p=128)
            )
            w2_sb = w2pool.tile([128, KT2, H2], BF16, tag="w2")
            nc.sync.dma_start(
                w2_sb[:, :, :], aps["w2"][e].rearrange("(t p) o -> p t o", p=128)
            )

            # layer 1: hT[f, b] = relu(sum_d W1[d, f] * xT[d, b] + b1[f])
            ht_sb = htpool.tile([128, KT2, BL], BF16, tag="ht")
            for m in range(MT1):
                ps = l1ps.tile([128, BL], F32, tag="ps1")
                for kt in range(KT1):
                    nc.tensor.matmul(
                        ps[:, :],
                        lhsT=w1_sb[:, kt, bass.ts(m, 128)],
                        rhs=xtb_sb[:, kt, :],
                        start=(kt == 0), stop=(kt == KT1 - 1),
                    )
                bias1 = b1_sb[:, e * MT1 + m : e * MT1 + m + 1] if has_b1 else 0.0
                nc.scalar.activation(ht_sb[:, m, :], ps[:, :], AF.Relu, bias=bias1)

            # layer 2: out[b, o] = relu(sum_h hT[h, b] * W2[h, o] + b2[o])
            for mb in range(NB):
                for no in range(NO):
                    ps2 = l2ps.tile([128, 512], F32, tag="ps2")
                    for kt in range(KT2):
                        nc.tensor.matmul(
                            ps2[:, :],
                            lhsT=ht_sb[:, kt, bass.ts(mb, 128)],
                            rhs=w2_sb[:, kt, bass.ts(no, 512)],
                            start=(kt == 0),
                            stop=(kt == KT2 - 1 and not has_b2),
                        )
                    if has_b2:
                        nc.tensor.matmul(
                            ps2[:, :],
                            lhsT=ones_sb[:, :],
                            rhs=b2_sb[:, e * H2 + no * 512 : e * H2 + (no + 1) * 512],
                            start=False, stop=True,
                        )
                    gcol = gate_sb[:, mb, e : e + 1]
                    if e == 0:
                        nc.scalar.activation(
                            acc_sb[:, mb, bass.ts(no, 512)], ps2[:, :], AF.Relu, scale=gcol
                        )
                    else:
                        tmp = tmppool.tile([128, 512], F32, tag="tmp")
                        nc.scalar.activation(tmp[:, :], ps2[:, :], AF.Relu, scale=gcol)
                        nc.vector.tensor_tensor(
                            out=acc_sb[:, mb, bass.ts(no, 512)],
                            in0=acc_sb[:, mb, bass.ts(no, 512)],
                            in1=tmp[:, :], op=ALU.add,
                        )

        nc.sync.dma_start(aps["out"].rearrange("(t p) o -> p t o", p=128), acc_sb[:, :, :])


def build(has_b1, has_b2):
    """Build + schedule + compile the Bass program. Returns nc."""
    nc = bacc.Bacc("TRN2", target_bir_lowering=False, debug=False)
    aps = {}
    aps["xT"] = nc.dram_tensor("xT", [D, BL], F32, kind="ExternalInput").ap()
    aps["xTb"] = nc.dram_tensor("xTb", [D, BL], BF16, kind="ExternalInput").ap()
    aps["w1"] = nc.dram_tensor("w1", [E, D, H1], BF16, kind="ExternalInput").ap()
    aps["w2"] = nc.dram_tensor("w2", [E, H1, H2], BF16, kind="ExternalInput").ap()
    if has_b1:
        aps["b1t"] = nc.dram_tensor("b1t", [128, E * MT1], F32, kind="ExternalInput").ap()
    if has_b2:
        aps["b2f"] = nc.dram_tensor("b2f", [1, E * H2], BF16, kind="ExternalInput").ap()
    aps["sflat"] = nc.dram_tensor("sflat", [D, EN], F32, kind="ExternalInput").ap()
    aps["sett"] = nc.dram_tensor("sett", [10, EN], F32, kind="ExternalInput").ap()
    aps["scol"] = nc.dram_tensor("scol", [128, NB], F32, kind="ExternalInput").ap()
    aps["srow"] = nc.dram_tensor("srow", [1, BL], F32, kind="ExternalInput").ap()
    aps["iota7"] = nc.dram_tensor("iota7", [1, EN], F32, kind="ExternalInput").ap()
    aps["iota10"] = nc.dram_tensor("iota10", [10, 1], F32, kind="ExternalInput").ap()
    aps["out"] = nc.dram_tensor("out", [BL, H2], F32, kind="ExternalOutput").ap()

    with tile.TileContext(nc) as tc:
        _emit_kernel(tc, aps, has_b1, has_b2)
    nc.compile()
    return nc


def make_in_maps(inputs):
    """Host-side layout prep + batch sharding. Returns (in_maps, has_b1, has_b2)."""
    x = np.ascontiguousarray(np.asarray(inputs["x"], dtype=np.float32))
    scene = np.asarray(inputs["scene"]).astype(np.int64)
    W1 = np.asarray(inputs["W1"], dtype=np.float32)
    b1 = np.asarray(inputs["b1"], dtype=np.float32)
    W2 = np.asarray(inputs["W2"], dtype=np.float32)
    b2 = np.asarray(inputs["b2"], dtype=np.float32)
    S = np.asarray(inputs["S"], dtype=np.float32)
    scene_emb = np.asarray(inputs["scene_emb"], dtype=np.float32)

    has_b1 = bool(np.any(b1))
    has_b2 = bool(np.any(b2))

    w1b = np.ascontiguousarray(W1.astype(NP_BF16))
    w2b = np.ascontiguousarray(W2.astype(NP_BF16))
    sflat = np.ascontiguousarray(S[:, :D, :].transpose(1, 2, 0).reshape(D, EN))
    sett = np.ascontiguousarray(
        np.einsum("rm,sme->res", scene_emb, S[:, D:, :]).reshape(scene_emb.shape[0], EN)
    )
    iota7 = np.arange(EN, dtype=np.float32).reshape(1, EN) % NS
    iota10 = np.arange(10, dtype=np.float32).reshape(10, 1)
    shared = {
        "w1": w1b, "w2": w2b, "sflat": sflat, "sett": sett,
        "iota7": iota7, "iota10": iota10,
    }
    if has_b1:
        shared["b1t"] = np.ascontiguousarray(
            b1.reshape(E, MT1, 128).transpose(2, 0, 1).reshape(128, E * MT1)
        )
    if has_b2:
        shared["b2f"] = np.ascontiguousarray(b2.astype(NP_BF16).reshape(1, E * H2))

    in_maps = []
    for c in range(N_CORES):
        xs = x[c * BL : (c + 1) * BL]
        sc = scene[c * BL : (c + 1) * BL]
        xT = np.ascontiguousarray(xs.T)
        m = dict(shared)
        m["xT"] = xT
        m["xTb"] = np.ascontiguousarray(xT.astype(NP_BF16))
        m["scol"] = np.ascontiguousarray(sc.reshape(NB, 128).T.astype(np.float32))
        m["srow"] = np.ascontiguousarray(sc.astype(np.float32).reshape(1, BL))
        in_maps.append(m)
    return in_maps, has_b1, has_b2


_NC_CACHE = {}


def get_compiled(has_b1, has_b2):
    key = (has_b1, has_b2)
    if key not in _NC_CACHE:
        _NC_CACHE[key] = build(has_b1, has_b2)
    return _NC_CACHE[key]


def run(inputs, trace=False, **kwargs):
    """Run on hardware; returns (full_output, BassKernelResults)."""
    in_maps, has_b1, has_b2 = make_in_maps(inputs)
    nc = get_compiled(has_b1, has_b2)
    res = run_bass_kernel_spmd(nc, in_maps, core_ids=list(range(N_CORES)), trace=trace, **kwargs)
    parts = [res.results[c]["out"] for c in range(N_CORES)]
    out = np.concatenate(parts, axis=0).astype(np.float32)
    full = np.ascontiguousarray(np.broadcast_to(out[None], (T, B, H2)))
    return full, res


def kernel(**inputs):
    full, _ = run(inputs, trace=False)
    return full


# revision 6
# speedup vs baseline: 1.0247x; 1.0247x over previous
"""MMoE layer kernel for 8 Trainium2 NeuronCores.

Reference math (B=4096, D=1024, H1=2048, H2=1024, E=7 experts, NS=7 scenes):
  h        = relu(einsum('bd,edh', x, W1) + b1)           # [B,E,H1]
  eo       = relu(einsum('beh,eho', h, W2) + b2)          # [B,E,H2]
  xc       = concat(x, scene_emb[scene])                  # [B, D+16]
  G        = softmax over s of einsum('bd,sde', xc, S)    # [B,E,NS] (after transpose)
  q        = mean_s log(G*7)                              # [B,E]
  score1   = logG[b, e, scene_b]
  select   = drop expert e iff e == argmin_e score1 == argmin_e q
  gate     = softmax_e(G[b,e,scene_b]) * select
  out      = einsum('be,beo', gate, eo); output = stack([out, out])

Sharding: data-parallel over batch (512 rows/core), weights replicated.
Expert MLP matmuls run in bf16 (fp32 accumulation in PSUM); all routing
math stays fp32 so the argmin/select decisions are bit-stable.

Device decomposition of the routing (no cross-partition broadcasts):
  Gpre[b, e*7+s] = x[b] @ Sflat + SE_table[scene_b]   (SE_table = scene_emb @ S[:,D:,:])
  Z = sum_s exp(Gpre); logZ = ln Z; SG = sum_s Gpre
  q      = SG/7 - logZ            (+const, argmin only)
  score1 = sum_s Gpre*onehot_s(scene) - logZ
  gate0  = softmax_e(exp(score1)) (logits in (0,1): no max-subtract needed)
  sel    = 1 - ismin(score1)*ismin(q)
  gate   = gate0 * sel
"""

import sys

if "/opt/trn_rl_repo" not in sys.path:
    sys.path.insert(0, "/opt/trn_rl_repo")

from contextlib import ExitStack

import ml_dtypes
import numpy as np

import concourse.bass as bass
import concourse.tile as tile
from concourse import bacc, mybir
from concourse.bass_utils import run_bass_kernel_spmd

F32 = mybir.dt.float32
BF16 = mybir.dt.bfloat16
AF = mybir.ActivationFunctionType
ALU = mybir.AluOpType
AX = mybir.AxisListType

N_CORES = 8
B, D, H1, H2, E, NS, T = 4096, 1024, 2048, 1024, 7, 7, 2
BL = B // N_CORES          # 512 rows per core
NB = BL // 128             # 4 batch tiles
KT1 = D // 128             # 8  k-tiles, layer 1
MT1 = H1 // 128            # 16 m-tiles, layer 1
KT2 = H1 // 128            # 16 k-tiles, layer 2
NO = H2 // 512             # 2  512-wide out column blocks
EN = E * NS                # 49
NP_BF16 = np.dtype(ml_dtypes.bfloat16)


def _emit_kernel(tc, aps, has_b1, has_b2):
    nc = tc.nc
    ctx = ExitStack()
    with ctx:
        consts = ctx.enter_context(tc.tile_pool(name="consts", bufs=1))

        # ---- routing inputs first: the routing matmuls only need these,
        # so they reach the PE early (warming HAM) while the first expert's
        # weights stream in right behind on the same HWDGE queue. ----------
        rpool = tc.alloc_tile_pool(name="routing", bufs=1)
        rps = tc.alloc_tile_pool(name="rps", bufs=2, space="PSUM")

        xt_sb = rpool.tile([128, KT1, BL], F32)
        nc.sync.dma_start(xt_sb[:, :, :], aps["xT"].rearrange("(t p) b -> p t b", p=128))
        sflat_sb = rpool.tile([128, KT1, EN], F32)
        nc.sync.dma_start(sflat_sb[:, :, :], aps["sflat"].rearrange("(t p) j -> p t j", p=128))
        sett_sb = rpool.tile([10, EN], F32)
        nc.sync.dma_start(sett_sb[:, :], aps["sett"][:, :])
        scol_sb = rpool.tile([128, NB], F32)
        nc.sync.dma_start(scol_sb[:, :], aps["scol"][:, :])
        srow10_sb = rpool.tile([10, BL], F32)
        nc.sync.dma_start(srow10_sb[:, :], aps["srow"].to_broadcast((10, BL)))
        io7_sb = rpool.tile([128, EN], F32)
        nc.sync.dma_start(io7_sb[:, :], aps["iota7"].to_broadcast((128, EN)))
        io10_sb = rpool.tile([10, 1], F32)
        nc.sync.dma_start(io10_sb[:, :], aps["iota10"][:, :])

        # ---- persistent SBUF state -------------------------------------
        xtb_sb = consts.tile([128, KT1, BL], BF16)
        nc.sync.dma_start(xtb_sb[:, :, :], aps["xTb"].rearrange("(t p) b -> p t b", p=128))
        gate_sb = consts.tile([128, NB, E], F32)
        acc_sb = consts.tile([128, NB, H2], F32)
        if has_b1:
            b1_sb = consts.tile([128, E * MT1], F32)
            nc.sync.dma_start(b1_sb[:, :], aps["b1t"][:, :])
        if has_b2:
            b2_sb = consts.tile([1, E * H2], BF16)
            nc.sync.dma_start(b2_sb[:, :], aps["b2f"][:, :])
            ones_sb = consts.tile([1, 128], BF16)
            nc.vector.memset(ones_sb[:, :], 1.0)

        # onehot over embedding rows, [10, BL]: onehot[r, b] = (scene[b] == r)
        onehot_sb = rpool.tile([10, BL], F32)
        nc.vector.tensor_scalar(
            out=onehot_sb[:, :], in0=srow10_sb[:, :],
            scalar1=io10_sb[:, 0:1], scalar2=None, op0=ALU.is_equal,
        )

        for t in range(NB):
            psr = rps.tile([128, EN], F32, tag="rps")
            for kt in range(KT1):
                nc.tensor.matmul(
                    psr[:, :],
                    lhsT=xt_sb[:, kt, bass.ts(t, 128)],
                    rhs=sflat_sb[:, kt, :],
                    start=(kt == 0), stop=False,
                )
            nc.tensor.matmul(
                psr[:, :],
                lhsT=onehot_sb[:, bass.ts(t, 128)],
                rhs=sett_sb[:, :],
                start=False, stop=True,
            )
            gp = rpool.tile([128, EN], F32, tag="gp")
            nc.scalar.copy(gp[:, :], psr[:, :])
            gp3 = gp.rearrange("p (e s) -> p e s", s=NS)

            eex = rpool.tile([128, EN], F32, tag="eex")
            nc.scalar.activation(eex[:, :], gp[:, :], AF.Exp)
            z = rpool.tile([128, E], F32, tag="z")
            nc.vector.tensor_reduce(out=z[:, :], in_=eex.rearrange("p (e s) -> p e s", s=NS), axis=AX.X, op=ALU.add)
            logz = rpool.tile([128, E], F32, tag="logz")
            nc.scalar.activation(logz[:, :], z[:, :], AF.Ln)
            sg = rpool.tile([128, E], F32, tag="sg")
            nc.vector.tensor_reduce(out=sg[:, :], in_=gp3, axis=AX.X, op=ALU.add)
            q = rpool.tile([128, E], F32, tag="q")
            nc.vector.scalar_tensor_tensor(
                out=q[:, :], in0=sg[:, :], scalar=1.0 / NS, in1=logz[:, :],
                op0=ALU.mult, op1=ALU.subtract,
            )
            oh49 = rpool.tile([128, EN], F32, tag="oh49")
            nc.vector.tensor_scalar(
                out=oh49[:, :], in0=io7_sb[:, :],
                scalar1=scol_sb[:, t : t + 1], scalar2=None, op0=ALU.is_equal,
            )
            gsel = rpool.tile([128, EN], F32, tag="gsel")
            nc.vector.tensor_tensor(out=gsel[:, :], in0=gp[:, :], in1=oh49[:, :], op=ALU.mult)
            s1s = rpool.tile([128, E], F32, tag="s1s")
            nc.vector.tensor_reduce(out=s1s[:, :], in_=gsel.rearrange("p (e s) -> p e s", s=NS), axis=AX.X, op=ALU.add)
            score1 = rpool.tile([128, E], F32, tag="score1")
            nc.vector.tensor_tensor(out=score1[:, :], in0=s1s[:, :], in1=logz[:, :], op=ALU.subtract)

            lg = rpool.tile([128, E], F32, tag="lg")
            nc.scalar.activation(lg[:, :], score1[:, :], AF.Exp)     # G at scene, in (0,1)
            el = rpool.tile([128, E], F32, tag="el")
            nc.scalar.activation(el[:, :], lg[:, :], AF.Exp)         # softmax numerator
            ssum = rpool.tile([128, 1], F32, tag="ssum")
            nc.vector.tensor_reduce(out=ssum[:, :], in_=el[:, :], axis=AX.X, op=ALU.add)
            rs = rpool.tile([128, 1], F32, tag="rs")
            nc.vector.reciprocal(rs[:, :], ssum[:, :])

            m1 = rpool.tile([128, 1], F32, tag="m1")
            nc.vector.tensor_reduce(out=m1[:, :], in_=score1[:, :], axis=AX.X, op=ALU.min)
            m2 = rpool.tile([128, 1], F32, tag="m2")
            nc.vector.tensor_reduce(out=m2[:, :], in_=q[:, :], axis=AX.X, op=ALU.min)
            k1 = rpool.tile([128, E], F32, tag="k1")
            nc.vector.tensor_scalar(
                out=k1[:, :], in0=score1[:, :], scalar1=m1[:, 0:1], scalar2=None, op0=ALU.is_equal,
            )
            k2 = rpool.tile([128, E], F32, tag="k2")
            nc.vector.tensor_scalar(
                out=k2[:, :], in0=q[:, :], scalar1=m2[:, 0:1], scalar2=None, op0=ALU.is_equal,
            )
            kill = rpool.tile([128, E], F32, tag="kill")
            nc.vector.tensor_tensor(out=kill[:, :], in0=k1[:, :], in1=k2[:, :], op=ALU.mult)
            sel = rpool.tile([128, E], F32, tag="sel")
            nc.vector.tensor_scalar(
                out=sel[:, :], in0=kill[:, :], scalar1=-1.0, scalar2=1.0,
                op0=ALU.mult, op1=ALU.add,
            )
            g0 = rpool.tile([128, E], F32, tag="g0")
            nc.vector.tensor_scalar(
                out=g0[:, :], in0=el[:, :], scalar1=rs[:, 0:1], scalar2=None, op0=ALU.mult,
            )
            nc.vector.tensor_tensor(out=gate_sb[:, t, :], in0=g0[:, :], in1=sel[:, :], op=ALU.mult)

        rps.release()
        rpool.release()

        # ---- expert MLPs (bf16 matmuls, fp32 accumulation) -------------
        w1pool = ctx.enter_context(tc.tile_pool(name="w1", bufs=2))
        w2pool = ctx.enter_context(tc.tile_pool(name="w2", bufs=2))
        htpool = ctx.enter_context(tc.tile_pool(name="ht", bufs=1))
        tmppool = ctx.enter_context(tc.tile_pool(name="tmp", bufs=3))
        l1ps = ctx.enter_context(tc.tile_pool(name="l1ps", bufs=4, space="PSUM"))
        l2ps = ctx.enter_context(tc.tile_pool(name="l2ps", bufs=4, space="PSUM"))

        for e in range(E):
            # Per-k-slice DMAs: the first expert's L1 matmuls can start as
            # soon as slice 0 lands instead of waiting for the full 4MB.
            w1_sb = w1pool.tile([128, KT1, H1], BF16, tag="w1")
            w1_src = aps["w1"][e].rearrange("(t p) h -> p t h", p=128)
            for kt in range(KT1):
                nc.sync.dma_start(w1_sb[:, kt, :], w1_src[:, kt, :])
            w2_sb = w2pool.tile([128, KT2, H2], BF16, tag="w2")
            nc.sync.dma_start(
                w2_sb[:, :, :], aps["w2"][e].rearrange("(t p) o -> p t o", p=128)
            )

            # layer 1: hT[f, b] = relu(sum_d W1[d, f] * xT[d, b] + b1[f])
            ht_sb = htpool.tile([128, KT2, BL], BF16, tag="ht")
            for m in range(MT1):
                ps = l1ps.tile([128, BL], F32, tag="ps1")
                for kt in range(KT1):
                    nc.tensor.matmul(
                        ps[:, :],
                        lhsT=w1_sb[:, kt, bass.ts(m, 128)],
                        rhs=xtb_sb[:, kt, :],
                        start=(kt == 0), stop=(kt == KT1 - 1),
                    )
                bias1 = b1_sb[:, e * MT1 + m : e * MT1 + m + 1] if has_b1 else 0.0
                nc.scalar.activation(ht_sb[:, m, :], ps[:, :], AF.Relu, bias=bias1)

            # layer 2: out[b, o] = relu(sum_h hT[h, b] * W2[h, o] + b2[o])
            for mb in range(NB):
                for no in range(NO):
                    ps2 = l2ps.tile([128, 512], F32, tag="ps2")
                    for kt in range(KT2):
                        nc.tensor.matmul(
                            ps2[:, :],
                            lhsT=ht_sb[:, kt, bass.ts(mb, 128)],
                            rhs=w2_sb[:, kt, bass.ts(no, 512)],
                            start=(kt == 0),
                            stop=(kt == KT2 - 1 and not has_b2),
                        )
                    if has_b2:
                        nc.tensor.matmul(
                            ps2[:, :],
                            lhsT=ones_sb[:, :],
                            rhs=b2_sb[:, e * H2 + no * 512 : e * H2 + (no + 1) * 512],
                            start=False, stop=True,
                        )
                    gcol = gate_sb[:, mb, e : e + 1]
                    if e == 0:
                        nc.scalar.activation(
                            acc_sb[:, mb, bass.ts(no, 512)], ps2[:, :], AF.Relu, scale=gcol
                        )
                    else:
                        tmp = tmppool.tile([128, 512], F32, tag="tmp")
                        nc.scalar.activation(tmp[:, :], ps2[:, :], AF.Relu, scale=gcol)
                        nc.vector.tensor_tensor(
                            out=acc_sb[:, mb, bass.ts(no, 512)],
                            in0=acc_sb[:, mb, bass.ts(no, 512)],
                            in1=tmp[:, :], op=ALU.add,
                        )
                # Per-batch-tile output DMA so the store overlaps the
                # remaining experts' compute instead of tailing the kernel.
                if e == E - 1:
                    nc.sync.dma_start(
                        aps["out"].rearrange("(t p) o -> p t o", p=128)[:, mb, :],
                        acc_sb[:, mb, :],
                    )


def build(has_b1, has_b2):
    """Build + schedule + compile the Bass program. Returns nc."""
    nc = bacc.Bacc("TRN2", target_bir_lowering=False, debug=False)
    aps = {}
    aps["xT"] = nc.dram_tensor("xT", [D, BL], F32, kind="ExternalInput").ap()
    aps["xTb"] = nc.dram_tensor("xTb", [D, BL], BF16, kind="ExternalInput").ap()
    aps["w1"] = nc.dram_tensor("w1", [E, D, H1], BF16, kind="ExternalInput").ap()
    aps["w2"] = nc.dram_tensor("w2", [E, H1, H2], BF16, kind="ExternalInput").ap()
    if has_b1:
        aps["b1t"] = nc.dram_tensor("b1t", [128, E * MT1], F32, kind="ExternalInput").ap()
    if has_b2:
        aps["b2f"] = nc.dram_tensor("b2f", [1, E * H2], BF16, kind="ExternalInput").ap()
    aps["sflat"] = nc.dram_tensor("sflat", [D, EN], F32, kind="ExternalInput").ap()
    aps["sett"] = nc.dram_tensor("sett", [10, EN], F32, kind="ExternalInput").ap()
    aps["scol"] = nc.dram_tensor("scol", [128, NB], F32, kind="ExternalInput").ap()
    aps["srow"] = nc.dram_tensor("srow", [1, BL], F32, kind="ExternalInput").ap()
    aps["iota7"] = nc.dram_tensor("iota7", [1, EN], F32, kind="ExternalInput").ap()
    aps["iota10"] = nc.dram_tensor("iota10", [10, 1], F32, kind="ExternalInput").ap()
    aps["out"] = nc.dram_tensor("out", [BL, H2], F32, kind="ExternalOutput").ap()

    with tile.TileContext(nc) as tc:
        _emit_kernel(tc, aps, has_b1, has_b2)
    nc.compile()
    return nc


def make_in_maps(inputs):
    """Host-side layout prep + batch sharding. Returns (in_maps, has_b1, has_b2)."""
    x = np.ascontiguousarray(np.asarray(inputs["x"], dtype=np.float32))
    scene = np.asarray(inputs["scene"]).astype(np.int64)
    W1 = np.asarray(inputs["W1"], dtype=np.float32)
    b1 = np.asarray(inputs["b1"], dtype=np.float32)
    W2 = np.asarray(inputs["W2"], dtype=np.float32)
    b2 = np.asarray(inputs["b2"], dtype=np.float32)
    S = np.asarray(inputs["S"], dtype=np.float32)
    scene_emb = np.asarray(inputs["scene_emb"], dtype=np.float32)

    has_b1 = bool(np.any(b1))
    has_b2 = bool(np.any(b2))

    w1b = np.ascontiguousarray(W1.astype(NP_BF16))
    w2b = np.ascontiguousarray(W2.astype(NP_BF16))
    sflat = np.ascontiguousarray(S[:, :D, :].transpose(1, 2, 0).reshape(D, EN))
    sett = np.ascontiguousarray(
        np.einsum("rm,sme->res", scene_emb, S[:, D:, :]).reshape(scene_emb.shape[0], EN)
    )
    iota7 = np.arange(EN, dtype=np.float32).reshape(1, EN) % NS
    iota10 = np.arange(10, dtype=np.float32).reshape(10, 1)
    shared = {
        "w1": w1b, "w2": w2b, "sflat": sflat, "sett": sett,
        "iota7": iota7, "iota10": iota10,
    }
    if has_b1:
        shared["b1t"] = np.ascontiguousarray(
            b1.reshape(E, MT1, 128).transpose(2, 0, 1).reshape(128, E * MT1)
        )
    if has_b2:
        shared["b2f"] = np.ascontiguousarray(b2.astype(NP_BF16).reshape(1, E * H2))

    in_maps = []
    for c in range(N_CORES):
        xs = x[c * BL : (c + 1) * BL]
        sc = scene[c * BL : (c + 1) * BL]
        xT = np.ascontiguousarray(xs.T)
        m = dict(shared)
        m["xT"] = xT
        m["xTb"] = np.ascontiguousarray(xT.astype(NP_BF16))
        m["scol"] = np.ascontiguousarray(sc.reshape(NB, 128).T.astype(np.float32))
        m["srow"] = np.ascontiguousarray(sc.astype(np.float32).reshape(1, BL))
        in_maps.append(m)
    return in_maps, has_b1, has_b2


_NC_CACHE = {}


def get_compiled(has_b1, has_b2):
    key = (has_b1, has_b2)
    if key not in _NC_CACHE:
        _NC_CACHE[key] = build(has_b1, has_b2)
    return _NC_CACHE[key]


def run(inputs, trace=False, **kwargs):
    """Run on hardware; returns (full_output, BassKernelResults)."""
    in_maps, has_b1, has_b2 = make_in_maps(inputs)
    nc = get_compiled(has_b1, has_b2)
    res = run_bass_kernel_spmd(nc, in_maps, core_ids=list(range(N_CORES)), trace=trace, **kwargs)
    parts = [res.results[c]["out"] for c in range(N_CORES)]
    out = np.concatenate(parts, axis=0).astype(np.float32)
    full = np.ascontiguousarray(np.broadcast_to(out[None], (T, B, H2)))
    return full, res


def kernel(**inputs):
    full, _ = run(inputs, trace=False)
    return full
